# revision 2
# baseline (speedup 1.0000x reference)
"""Trainium2 Bass kernel for nn_CopyMechanismMixin (copy-mechanism + vocab projection).

Sharding: vocab-parallel across 8 cores for the dominant [1024tok,1024]x[1024,50257]
projection + softmax + scatter; token-parallel (1 batch x 128 tokens per core) for
the small copy-attention prologue. Cross-core: one AllGather (copy probs, bf16) and
one AllReduce (softmax denominators, f32). Scatter-add of copy probs into the vocab
distribution is done with indirect-DMA row gather/scatter on the output table in
[vocab, token] layout, after merging duplicate vocab ids with is_equal selection
matmuls.
"""

import numpy as np
import ml_dtypes

import concourse.bass as bass
import concourse.bacc as bacc
import concourse.mybir as mybir
import concourse.tile as tile
from concourse.bass_utils import run_bass_kernel_spmd
from concourse.masks import make_identity

F32 = mybir.dt.float32
BF16 = mybir.dt.bfloat16
I32 = mybir.dt.int32
BF = ml_dtypes.bfloat16
AF = mybir.ActivationFunctionType
ALU = mybir.AluOpType

B, T, M, D, V = 4, 256, 512, 1024, 50257
NCORES = 8
VS = -(-V // NCORES)          # 6283 per-core vocab shard
NVT = -(-VS // 128)           # 50 vocab tiles per core
VP = NVT * 128                # 6400 padded shard width
SENT = VP - 1                 # sentinel row (pad region)
PAD_BIAS = -30.0              # b_out value for pad rows -> exp ~ 1e-13
MASK_NEG = -30000.0           # additive score mask
NT = B * T                    # 1024 total tokens
TC = 128                      # tokens per core (attention phase)
KD = D // 128                 # 8 contraction chunks of 128
KE = 2 * D // 128             # 16


def build_kernel():
    nc = bacc.Bacc(
        "TRN2",
        target_bir_lowering=False,
        debug=False,
        enable_asserts=False,
        num_devices=NCORES,
    )
    # ---- I/O ----
    dec_myT = nc.dram_tensor("dec_myT", [D, TC], F32, kind="ExternalInput")
    dec_myT_bf = nc.dram_tensor("dec_myT_bf", [D, TC], BF16, kind="ExternalInput")
    decT = nc.dram_tensor("decT", [KD, 128, NT], BF16, kind="ExternalInput")
    wcopy = nc.dram_tensor("wcopy", [D, D], F32, kind="ExternalInput")
    wdecT = nc.dram_tensor("wdecT", [2 * D, D], BF16, kind="ExternalInput")
    wgenT = nc.dram_tensor("wgenT", [D, 1], BF16, kind="ExternalInput")
    bdec = nc.dram_tensor("bdec", [D, 1], F32, kind="ExternalInput")
    bgen = nc.dram_tensor("bgen", [128, 1], F32, kind="ExternalInput")
    membT = nc.dram_tensor("membT", [D, M], F32, kind="ExternalInput")
    memb = nc.dram_tensor("memb", [M, D], F32, kind="ExternalInput")
    maskb = nc.dram_tensor("maskb", [TC, M], F32, kind="ExternalInput")
    woutT = nc.dram_tensor("woutT", [NVT, KD, 128, 128], BF16, kind="ExternalInput")
    bo = nc.dram_tensor("bo", [128, NVT], F32, kind="ExternalInput")
    ids_f = nc.dram_tensor("ids_f", [B, 4, 128, 1], F32, kind="ExternalInput")
    ids_loc = nc.dram_tensor("ids_loc", [B, 4, 128, 1], I32, kind="ExternalInput")
    outb = [
        nc.dram_tensor(f"out{b}", [VP, T], F32, kind="ExternalOutput") for b in range(B)
    ]

    with tile.TileContext(nc) as tc:
        with (
            tc.tile_pool(name="const", bufs=1) as constp,
            tc.tile_pool(name="wstream", bufs=4) as wsp,
            tc.tile_pool(name="attn", bufs=1) as ap,
            tc.tile_pool(name="small", bufs=2) as sp,
            tc.tile_pool(name="stash", bufs=54) as stashp,
            tc.tile_pool(name="wout", bufs=3) as wop,
            tc.tile_pool(name="fin", bufs=3) as finp,
            tc.tile_pool(name="mrg", bufs=2) as mrgp,
            tc.tile_pool(name="psA", bufs=2, space="PSUM") as psA,
            tc.tile_pool(name="psB", bufs=2, space="PSUM") as psB,
            tc.tile_pool(name="psS", bufs=2, space="PSUM") as psS,
            tc.tile_pool(name="dram", bufs=1, space="DRAM") as dram,
        ):
            ident = constp.tile([128, 128], F32, tag="ident")
            make_identity(nc, ident[:])

            # ============ Phase A: copy-attention for my 128 tokens ============
            # dec_myT (f32) -> SBUF [128, KD*128]
            dmt = constp.tile([128, KD * 128], F32, tag="dmt")
            for kd in range(KD):
                nc.sync.dma_start(
                    out=dmt[:, kd * 128 : (kd + 1) * 128],
                    in_=dec_myT[kd * 128 : (kd + 1) * 128, :],
                )
            dmt_bf = constp.tile([128, KD * 128], BF16, tag="dmtbf")
            for kd in range(KD):
                nc.sync.dma_start(
                    out=dmt_bf[:, kd * 128 : (kd + 1) * 128],
                    in_=dec_myT_bf[kd * 128 : (kd + 1) * 128, :],
                )

            # dprojT[d, t] = sum_e W_copy[e, d] * decT[e, t]
            dpT = constp.tile([128, KD * 128], F32, tag="dpT")
            for dc in range(KD):
                ps = psA.tile([128, 128], F32, space="PSUM", tag="psa")
                for ke in range(KD):
                    wct = wsp.tile([128, 128], F32, tag="wc")
                    nc.sync.dma_start(
                        out=wct[:],
                        in_=wcopy[
                            ke * 128 : (ke + 1) * 128, dc * 128 : (dc + 1) * 128
                        ],
                    )
                    nc.tensor.matmul(
                        out=ps[:],
                        lhsT=wct[:],
                        rhs=dmt[:, ke * 128 : (ke + 1) * 128],
                        start=(ke == 0),
                        stop=(ke == KD - 1),
                    )
                nc.scalar.copy(dpT[:, dc * 128 : (dc + 1) * 128], ps[:])

            # scores[t, m] = sum_d dprojT[d, t] * memT[d, m]
            scps = psB.tile([128, M], F32, space="PSUM", tag="psb")
            for dc in range(KD):
                mTt = wsp.tile([128, M], F32, tag="mT")
                nc.sync.dma_start(
                    out=mTt[:], in_=membT[dc * 128 : (dc + 1) * 128, :]
                )
                nc.tensor.matmul(
                    out=scps[:],
                    lhsT=dpT[:, dc * 128 : (dc + 1) * 128],
                    rhs=mTt[:],
                    start=(dc == 0),
                    stop=(dc == KD - 1),
                )
            mbt = ap.tile([128, M], F32, tag="mbt")
            nc.sync.dma_start(out=mbt[:], in_=maskb[:])
            sc = ap.tile([128, M], F32, tag="sc")
            nc.vector.tensor_tensor(out=sc[:], in0=scps[:], in1=mbt[:], op=ALU.add)
            mx = sp.tile([128, 1], F32, tag="mx")
            nc.vector.reduce_max(out=mx[:], in_=sc[:], axis=mybir.AxisListType.X)
            nmx = sp.tile([128, 1], F32, tag="nmx")
            nc.vector.tensor_scalar_mul(nmx[:], mx[:], -1.0)
            esc = ap.tile([128, M], F32, tag="esc")
            sesum = sp.tile([128, 1], F32, tag="sesum")
            nc.scalar.activation(out=esc[:], in_=sc[:], func=AF.Exp, bias=nmx[:, :1])
            nc.vector.reduce_sum(out=sesum[:], in_=esc[:], axis=mybir.AxisListType.X)
            rinv = sp.tile([128, 1], F32, tag="rinv")
            nc.vector.reciprocal(rinv[:], sesum[:])
            attn = ap.tile([128, M], F32, tag="attn")
            nc.vector.tensor_scalar_mul(attn[:], esc[:], rinv[:, :1])

            # attnT via PE transpose -> [m-part x4, 128 t] f32
            aT = ap.tile([128, 4 * 128], F32, tag="aT")
            for mc in range(4):
                tp = psA.tile([128, 128], F32, space="PSUM", tag="psa")
                nc.tensor.transpose(
                    out=tp[:], in_=attn[:, mc * 128 : (mc + 1) * 128], identity=ident[:]
                )
                nc.scalar.copy(aT[:, mc * 128 : (mc + 1) * 128], tp[:])

            # attn_outT[d, t] = sum_m memb[m, d] * attnT[m, t]  -> bf16
            aoT_bf = ap.tile([128, KD * 128], BF16, tag="aoT")
            for dc in range(KD):
                ps = psA.tile([128, 128], F32, space="PSUM", tag="psa")
                for mc in range(4):
                    mbt2 = wsp.tile([128, 128], F32, tag="memb")
                    nc.sync.dma_start(
                        out=mbt2[:],
                        in_=memb[
                            mc * 128 : (mc + 1) * 128, dc * 128 : (dc + 1) * 128
                        ],
                    )
                    nc.tensor.matmul(
                        out=ps[:],
                        lhsT=mbt2[:],
                        rhs=aT[:, mc * 128 : (mc + 1) * 128],
                        start=(mc == 0),
                        stop=(mc == 3),
                    )
                nc.scalar.copy(aoT_bf[:, dc * 128 : (dc + 1) * 128], ps[:])

            # dwaT[d, t] = tanh(sum_e W_decT[e, d] * dec_catT[e, t] + b_dec[d]) bf16
            bd = constp.tile([128, KD], F32, tag="bd")
            nc.sync.dma_start(
                out=bd[:], in_=bdec[:].rearrange("(a p) o -> p (a o)", p=128)
            )
            th = ap.tile([128, KD * 128], BF16, tag="th")
            for dc in range(KD):
                ps = psA.tile([128, 128], F32, space="PSUM", tag="psa")
                for ec in range(KE):
                    wdt = wsp.tile([128, 128], BF16, tag="wd")
                    nc.sync.dma_start(
                        out=wdt[:],
                        in_=wdecT[
                            ec * 128 : (ec + 1) * 128, dc * 128 : (dc + 1) * 128
                        ],
                    )
                    rhs = (
                        dmt_bf[:, ec * 128 : (ec + 1) * 128]
                        if ec < KD
                        else aoT_bf[:, (ec - KD) * 128 : (ec - KD + 1) * 128]
                    )
                    nc.tensor.matmul(
                        out=ps[:], lhsT=wdt[:], rhs=rhs, start=(ec == 0), stop=(ec == KE - 1)
                    )
                nc.scalar.activation(
                    out=th[:, dc * 128 : (dc + 1) * 128],
                    in_=ps[:],
                    func=AF.Tanh,
                    bias=bd[:, dc : dc + 1],
                )

            # z[t] = sum_d W_gen[d] * dwaT[d, t] ; pg = sigmoid(z + b_gen)
            wg = constp.tile([128, KD], BF16, tag="wg")
            nc.sync.dma_start(
                out=wg[:], in_=wgenT[:].rearrange("(a p) o -> p (a o)", p=128)
            )
            zps = psA.tile([128, 128], F32, space="PSUM", tag="psa")
            for dc in range(KD):
                nc.tensor.matmul(
                    out=zps[:, :1],
                    lhsT=th[:, dc * 128 : (dc + 1) * 128],
                    rhs=wg[:, dc : dc + 1],
                    start=(dc == 0),
                    stop=(dc == KD - 1),
                )
            bg = constp.tile([128, 1], F32, tag="bg")
            nc.sync.dma_start(out=bg[:], in_=bgen[:])
            pg = sp.tile([128, 1], F32, tag="pg")
            nc.scalar.activation(
                out=pg[:], in_=zps[:, :1], func=AF.Sigmoid, bias=bg[:, :1]
            )
            # cp = esc * (rinv * (1 - pg))   (f32), then transpose+cast to bf16
            ompg = sp.tile([128, 1], F32, tag="ompg")
            nc.vector.tensor_scalar(
                out=ompg[:], in0=pg[:], scalar1=-1.0, scalar2=1.0, op0=ALU.mult, op1=ALU.add
            )
            s2 = sp.tile([128, 1], F32, tag="s2")
            nc.vector.tensor_tensor(out=s2[:], in0=rinv[:], in1=ompg[:], op=ALU.mult)
            cp = ap.tile([128, M], F32, tag="cp")
            nc.vector.tensor_scalar_mul(cp[:], esc[:], s2[:, :1])

            # AG contribution [M+1, 128] bf16: rows 0..511 cpT, row 512 pg
            ag_in = dram.tile([M + 1, TC], BF16)
            cpT_bf = ap.tile([128, 128], BF16, tag="cpTbf")
            for mc in range(4):
                tp = psA.tile([128, 128], F32, space="PSUM", tag="psa")
                nc.tensor.transpose(
                    out=tp[:], in_=cp[:, mc * 128 : (mc + 1) * 128], identity=ident[:]
                )
                nc.scalar.copy(cpT_bf[:], tp[:])
                nc.sync.dma_start(
                    out=ag_in[mc * 128 : (mc + 1) * 128, :], in_=cpT_bf[:]
                )
            pgpad = ap.tile([128, 128], F32, tag="pgpad")
            nc.vector.memset(pgpad[:], 0.0)
            nc.vector.tensor_copy(out=pgpad[:, 0:1], in_=pg[:])
            tp = psA.tile([128, 128], F32, space="PSUM", tag="psa")
            nc.tensor.transpose(out=tp[:], in_=pgpad[:], identity=ident[:])
            pgT_bf = sp.tile([1, 128], BF16, tag="pgT")
            nc.scalar.copy(pgT_bf[:], tp[0:1, :])
            nc.sync.dma_start(out=ag_in[M : M + 1, :], in_=pgT_bf[:])

            ag_out = dram.tile([NCORES * (M + 1), TC], BF16, addr_space="Shared")
            nc.gpsimd.collective_compute(
                "AllGather",
                ALU.bypass,
                replica_groups=[list(range(NCORES))],
                ins=[ag_in[:].opt()],
                outs=[ag_out[:].opt()],
            )

            # ===== Phase B: vocab-shard logits/softmax, 2 token rounds =====
            bos = constp.tile([128, NVT], F32, tag="bos")
            nc.sync.dma_start(out=bos[:], in_=bo[:])
            ones_bf = constp.tile([128, 1], BF16, tag="ones_bf")
            nc.vector.memset(ones_bf[:], 1.0)
            ones128 = constp.tile([128, 128], F32, tag="ones128")
            nc.vector.memset(ones128[:], 1.0)

            # pg row (all tokens) + cpT_all [m-part x4, NT] from AG output
            pgrow_bf = sp.tile([1, NT], BF16, tag="pgrowbf", bufs=1)
            for c in range(NCORES):
                nc.sync.dma_start(
                    out=pgrow_bf[:, c * TC : (c + 1) * TC],
                    in_=ag_out[c * (M + 1) + M : c * (M + 1) + M + 1, :],
                )
            pgrow = sp.tile([1, NT], F32, tag="pgrow", bufs=1)
            nc.vector.tensor_copy(out=pgrow[:], in_=pgrow_bf[:])
            cpT = constp.tile([128, 4 * NT], BF16, tag="cpT")
            for c in range(NCORES):
                for mc in range(4):
                    nc.sync.dma_start(
                        out=cpT[:, mc * NT + c * TC : mc * NT + (c + 1) * TC],
                        in_=ag_out[
                            c * (M + 1) + mc * 128 : c * (M + 1) + (mc + 1) * 128, :
                        ],
                    )

            for r in range(2):
                tok0 = r * 512
                da = wop.tile([128, KD * 512], BF16, tag="da", bufs=2, name=f"da{r}")
                for kd in range(KD):
                    nc.sync.dma_start(
                        out=da[:, kd * 512 : (kd + 1) * 512],
                        in_=decT[kd][:, tok0 : tok0 + 512],
                    )
                s_acc = sp.tile([1, 512], F32, tag="sacc", name=f"sacc{r}")
                nc.vector.memset(s_acc[:], 0.0)
                stash = []
                for vt in range(NVT):
                    wot = wop.tile(
                        [128, KD * 128], BF16, tag="wot", bufs=3, name=f"wot{r}_{vt}"
                    )
                    for kd in range(KD):
                        nc.sync.dma_start(
                            out=wot[:, kd * 128 : (kd + 1) * 128], in_=woutT[vt, kd]
                        )
                    st = stashp.tile(
                        [128, 512], BF16, tag="stash", name=f"st{r}_{vt}"
                    )
                    ps = psB.tile([128, 512], F32, space="PSUM", tag="psb", name="pslg")
                    for kd in range(KD):
                        nc.tensor.matmul(
                            out=ps[:],
                            lhsT=wot[:, kd * 128 : (kd + 1) * 128],
                            rhs=da[:, kd * 512 : (kd + 1) * 512],
                            start=(kd == 0),
                            stop=(kd == KD - 1),
                        )
                    nc.scalar.activation(
                        out=st[:], in_=ps[:], func=AF.Exp, bias=bos[:, vt : vt + 1]
                    )
                    spp = psS.tile(
                        [1, 512], F32, space="PSUM", tag="psS", name=f"spp{r}_{vt}"
                    )
                    nc.tensor.matmul(
                        out=spp[:], lhsT=ones_bf[:], rhs=st[:], start=True, stop=True
                    )
                    nc.vector.tensor_tensor(
                        out=s_acc[:], in0=s_acc[:], in1=spp[:], op=ALU.add
                    )
                    stash.append(st)

                ar_in = dram.tile([1, 512], F32, tag=f"ar_in{r}", name=f"ar_in{r}")
                ar_out = dram.tile(
                    [1, 512], F32, addr_space="Shared", tag=f"ar_out{r}", name=f"ar_out{r}"
                )
                nc.sync.dma_start(out=ar_in[:], in_=s_acc[:])
                nc.gpsimd.collective_compute(
                    "AllReduce",
                    ALU.add,
                    replica_groups=[list(range(NCORES))],
                    ins=[ar_in[:].opt()],
                    outs=[ar_out[:].opt()],
                )
                s_glob = sp.tile([1, 512], F32, tag="sglob", name=f"sglob{r}")
                nc.sync.dma_start(out=s_glob[:], in_=ar_out[:])
                sinv = sp.tile([1, 512], F32, tag="sinv", name=f"sinv{r}")
                nc.vector.reciprocal(sinv[:], s_glob[:])
                crow = sp.tile([1, 512], F32, tag="crow", name=f"crow{r}")
                nc.vector.tensor_tensor(
                    out=crow[:],
                    in0=pgrow[:, tok0 : tok0 + 512],
                    in1=sinv[:],
                    op=ALU.mult,
                )
                zc = finp.tile([128, 512], F32, tag="x", name=f"zc{r}")
                nc.vector.memset(zc[:], 0.0)
                nc.vector.tensor_copy(out=zc[0:1, :], in_=crow[:])
                psC = psB.tile([128, 512], F32, space="PSUM", tag="psb", name=f"psC{r}")
                nc.tensor.matmul(
                    out=psC[:], lhsT=ones128[:], rhs=zc[:], start=True, stop=True
                )
                Cbf = wop.tile([128, 512], BF16, tag="Cbf", bufs=2, name=f"Cbf{r}")
                nc.scalar.copy(Cbf[:], psC[:])

                for vt in range(NVT):
                    x = finp.tile([128, 512], F32, tag="x", name=f"x{r}_{vt}")
                    nc.vector.tensor_tensor(
                        out=x[:], in0=stash[vt][:], in1=Cbf[:], op=ALU.mult
                    )
                    y = finp.tile([128, 512], F32, tag="y", name=f"y{r}_{vt}")
                    nc.scalar.activation(out=y[:], in_=x[:], func=AF.Ln)
                    for bb in range(2):
                        nc.sync.dma_start(
                            out=outb[2 * r + bb][vt * 128 : (vt + 1) * 128, :],
                            in_=y[:, bb * T : (bb + 1) * T],
                        )

                # ===== scatter fixup for this round's two batches =====
                for bb in range(2):
                    b = 2 * r + bb
                    idf = mrgp.tile([128, 4], F32, tag="idf", name=f"idf{b}")
                    for mc in range(4):
                        nc.sync.dma_start(out=idf[:, mc : mc + 1], in_=ids_f[b, mc])
                    idT = mrgp.tile([128, 4 * 128], F32, tag="idT", name=f"idT{b}")
                    for mc in range(4):
                        tp = psA.tile(
                            [128, 128], F32, space="PSUM", tag="psa", name=f"tpi{b}_{mc}"
                        )
                        nc.tensor.transpose(
                            out=tp[:],
                            in_=idf[:, mc : mc + 1].to_broadcast([128, 128]),
                            identity=ident[:],
                        )
                        nc.scalar.copy(idT[:, mc * 128 : (mc + 1) * 128], tp[:])
                    mg = mrgp.tile([128, 4 * T], F32, tag="mg", name=f"mg{b}")
                    for mi in range(4):
                        ps = psA.tile(
                            [128, T], F32, space="PSUM", tag="psa", name=f"psm{b}_{mi}"
                        )
                        for mj in range(4):
                            sel = mrgp.tile(
                                [128, 128], BF16, tag="sel", name=f"sel{b}_{mi}_{mj}"
                            )
                            nc.vector.tensor_tensor(
                                out=sel[:],
                                in0=idf[:, mj : mj + 1].to_broadcast([128, 128]),
                                in1=idT[:, mi * 128 : (mi + 1) * 128],
                                op=ALU.is_equal,
                            )
                            nc.tensor.matmul(
                                out=ps[:],
                                lhsT=sel[:],
                                rhs=cpT[:, mj * NT + b * T : mj * NT + (b + 1) * T],
                                start=(mj == 0),
                                stop=(mj == 3),
                            )
                        nc.scalar.copy(mg[:, mi * T : (mi + 1) * T], ps[:])

                    for mc in range(4):
                        ilc = mrgp.tile([128, 1], I32, tag="ilc", name=f"ilc{b}_{mc}")
                        nc.sync.dma_start(out=ilc[:], in_=ids_loc[b, mc])
                        g = mrgp.tile([128, T], F32, tag="g", name=f"g{b}_{mc}")
                        nc.gpsimd.indirect_dma_start(
                            out=g[:],
                            out_offset=None,
                            in_=outb[b][:],
                            in_offset=bass.IndirectOffsetOnAxis(ap=ilc[:, :1], axis=0),
                        )
                        gx = mrgp.tile([128, T], F32, tag="gx", name=f"gx{b}_{mc}")
                        nc.scalar.activation(out=gx[:], in_=g[:], func=AF.Exp)
                        nc.vector.tensor_tensor(
                            out=gx[:],
                            in0=gx[:],
                            in1=mg[:, mc * T : (mc + 1) * T],
                            op=ALU.add,
                        )
                        gz = mrgp.tile([128, T], F32, tag="gz", name=f"gz{b}_{mc}")
                        nc.scalar.activation(out=gz[:], in_=gx[:], func=AF.Ln)
                        nc.gpsimd.indirect_dma_start(
                            out=outb[b][:],
                            out_offset=bass.IndirectOffsetOnAxis(ap=ilc[:, :1], axis=0),
                            in_=gz[:],
                            in_offset=None,
                        )
    nc.finalize()
    return nc


_NC_CACHE = {}


def _get_nc():
    if "nc" not in _NC_CACHE:
        _NC_CACHE["nc"] = build_kernel()
    return _NC_CACHE["nc"]


def kernel(
    decoder_output,
    memory_output,
    memory_sequence_length,
    memory_ids,
    W_copy,
    b_copy,
    W_dec,
    b_dec,
    W_gen,
    b_gen,
    W_out,
    b_out,
):
    decoder_output = np.asarray(decoder_output, dtype=np.float32)
    memory_output = np.asarray(memory_output, dtype=np.float32)
    msl = np.asarray(memory_sequence_length).astype(np.int64)
    ids = np.asarray(memory_ids).astype(np.int64)
    W_copy = np.asarray(W_copy, dtype=np.float32)
    W_dec = np.asarray(W_dec, dtype=np.float32)
    W_gen = np.asarray(W_gen, dtype=np.float32)
    b_dec_a = np.asarray(b_dec, dtype=np.float32)
    b_gen_a = np.asarray(b_gen, dtype=np.float32)
    W_out = np.asarray(W_out, dtype=np.float32)
    b_out_a = np.asarray(b_out, dtype=np.float32)
    # NOTE: b_copy drops out: it shifts scores by a per-token constant, which
    # softmax over the memory axis cancels exactly.

    # ---- shared (core-independent) host prep ----
    dec_flat = decoder_output.reshape(NT, D)  # token g = b*T + t
    decT_all = np.ascontiguousarray(dec_flat.T)  # [D, NT]
    decT_bf = np.ascontiguousarray(
        decT_all.reshape(KD, 128, NT).astype(BF)
    )  # [KD,128,NT]
    wdecT = np.ascontiguousarray(W_dec.T.astype(BF))  # [2D, D]
    wgenT = np.ascontiguousarray(W_gen.reshape(1, D).T.astype(BF))  # [D,1]
    bdec_h = np.ascontiguousarray(b_dec_a.reshape(D, 1))
    bgen_h = np.full((128, 1), float(b_gen_a.ravel()[0]), np.float32)
    ids_f_h = np.ascontiguousarray(
        ids.reshape(B, 4, 128, 1).astype(np.float32)
    )
    woutT_full = np.ascontiguousarray(W_out.T.astype(BF))  # [D, V]

    in_maps = []
    for c in range(NCORES):
        b = c // 2
        t0 = (c % 2) * TC
        v0 = c * VS
        v1 = min(v0 + VS, V)
        realw = v1 - v0

        dec_my = decoder_output[b, t0 : t0 + TC]  # [TC, D]
        dec_myT = np.ascontiguousarray(dec_my.T)  # [D, TC]
        membT_h = np.ascontiguousarray(memory_output[b].T)  # [D, M]
        memb_h = np.ascontiguousarray(memory_output[b])  # [M, D]
        L = int(msl[b])
        mrow = np.where(np.arange(M) < L, 0.0, MASK_NEG).astype(np.float32)
        maskb_h = np.ascontiguousarray(np.broadcast_to(mrow, (TC, M)))

        wt = np.zeros((D, VP), dtype=BF)
        wt[:, :realw] = woutT_full[:, v0:v1]
        woutT_h = np.ascontiguousarray(
            wt.reshape(D // 128, 128, NVT, 128).transpose(2, 0, 1, 3)
        )  # [NVT, KD, 128, 128]
        bo_pad = np.full(VP, PAD_BIAS, np.float32)
        bo_pad[:realw] = b_out_a[v0:v1]
        bo_h = np.ascontiguousarray(bo_pad.reshape(NVT, 128).T)  # [128, NVT]

        loc = ids - v0  # [B, M]
        valid = (ids >= v0) & (ids < v1) & (np.arange(M)[None, :] < msl[:, None])
        loc = np.where(valid, loc, SENT).astype(np.int32)
        # dedup: only the first occurrence of a vocab id per batch does the
        # RMW fixup (the selection-matmul merge already sums the whole group);
        # later occurrences would double-add.
        for bb_ in range(B):
            seen_ = set()
            for m_ in range(M):
                lv = int(loc[bb_, m_])
                if lv != SENT:
                    if lv in seen_:
                        loc[bb_, m_] = SENT
                    else:
                        seen_.add(lv)
        ids_loc_h = np.ascontiguousarray(loc.reshape(B, 4, 128, 1))

        in_maps.append(
            {
                "dec_myT": dec_myT,
                "dec_myT_bf": dec_myT.astype(BF),
                "decT": decT_bf,
                "wcopy": W_copy,
                "wdecT": wdecT,
                "wgenT": wgenT,
                "bdec": bdec_h,
                "bgen": bgen_h,
                "membT": membT_h,
                "memb": memb_h,
                "maskb": maskb_h,
                "woutT": woutT_h,
                "bo": bo_h,
                "ids_f": ids_f_h,
                "ids_loc": ids_loc_h,
            }
        )

    nc = _get_nc()
    import os

    trace = os.environ.get("KERNEL_TRACE") == "1"
    kw = {}
    if trace:
        kw["trace"] = True
        td = os.environ.get("KERNEL_TRACE_DIR")
        if td:
            os.makedirs(td, exist_ok=True)
            kw["tmpdir"] = td
        tcores = os.environ.get("KERNEL_TRACE_CORES")
        if tcores:
            kw["trace_cores"] = [int(x) for x in tcores.split(",")]
    res = run_bass_kernel_spmd(nc, in_maps, core_ids=list(range(NCORES)), **kw)
    global LAST
    LAST = res

    out_full = np.empty((V, B, T), np.float32)
    for c in range(NCORES):
        v0 = c * VS
        v1 = min(v0 + VS, V)
        realw = v1 - v0
        for b in range(B):
            out_full[v0:v1, b, :] = res.results[c][f"out{b}"][:realw, :]
    return np.ascontiguousarray(out_full.transpose(1, 2, 0))



# revision 15
# speedup vs baseline: 2.4471x; 2.4471x over previous
"""Trainium2 Bass kernel for nn_CopyMechanismMixin (copy-mechanism + vocab projection).

Sharding: vocab-parallel across 8 cores for the dominant [1024tok,1024]x[1024,50257]
projection + softmax + scatter; token-parallel (128 tokens per core) for the copy-
attention prologue. Cross-core: one AllGather (copy probs, bf16) and one AllReduce
(softmax denominators, f32) per 512-token round.

Perf structure:
- fp8(e4m3) DoubleRow matmuls for the vocab projection (weights pre-scaled x32,
  un-scaled in the activation that reads PSUM).
- Large-line DMA layouts: W_out streamed in 5 slabs/round of [128,10240] fp8
  (2KB contiguous per partition line); phase-A weights in [128,4096] bf16 slabs.
- Logits z are stashed in bf16; final log-prob y = z + C (C = ln(p_gen)-ln(S))
  is one vector add per tile; no ln over the bulk output.
- Softmax denominators accumulated on the vector engine (frees PE), one
  partition-sum matmul + AllReduce [1,512] per round.
- Output stored bf16 ([VP, 512] per round), converted to f32 on host.
- Scatter fixup: copy-prob mass merged per unique in-shard vocab id via
  selection matmuls into packed [128, T] tiles; one indirect row gather +
  exp/add/ln + one indirect scatter per batch.
"""

import numpy as np
import ml_dtypes

import concourse.bass as bass
import concourse.bacc as bacc
import concourse.mybir as mybir
import concourse.tile as tile
from concourse.bass_utils import run_bass_kernel_spmd
from concourse.masks import make_identity

F32 = mybir.dt.float32
BF16 = mybir.dt.bfloat16
FP8 = mybir.dt.float8e4
I32 = mybir.dt.int32
BF = ml_dtypes.bfloat16
F8 = ml_dtypes.float8_e4m3
AF = mybir.ActivationFunctionType
ALU = mybir.AluOpType
DR = mybir.MatmulPerfMode.DoubleRow

B, T, M, D, V = 4, 256, 512, 1024, 50257
NCORES = 8
VS = -(-V // NCORES)          # 6283 per-core vocab shard
NVT = -(-VS // 128)           # 50 vocab tiles per core
VP = NVT * 128                # 6400 padded shard width
NG = NVT // 2                 # 25 vt-pair groups in woutT layout
SENT = VP - 1                 # sentinel row (pad region)
PAD_BIAS = -30.0              # b_out value for pad rows -> exp ~ 1e-13
MASK_NEG = -30000.0           # additive score mask
WSCALE = 32.0                 # fp8 weight pre-scale
NT = B * T                    # 1024 total tokens
TC = 128                      # tokens per core (attention phase)
KD = D // 128                 # 8 contraction chunks of 128
KE = 2 * D // 128             # 16
RT = 512                      # tokens per round
NSLAB = 5                     # W_out slabs per round (10 vts each)
LAST = None


def build_kernel():
    nc = bacc.Bacc(
        "TRN2",
        target_bir_lowering=False,
        debug=False,
        enable_asserts=False,
        num_devices=NCORES,
    )
    # ---- I/O ----
    dmt_in = nc.dram_tensor("dmt_in", [128, KD * 128], BF16, kind="ExternalInput")
    wcs_in = nc.dram_tensor("wcs_in", [2, 128, 4 * KD * 128], BF16, kind="ExternalInput")
    wds_in = nc.dram_tensor("wds_in", [4, 128, 4 * KD * 128], BF16, kind="ExternalInput")
    membT_in = nc.dram_tensor("membT_in", [128, KD * 512], BF16, kind="ExternalInput")
    memb_in = nc.dram_tensor("memb_in", [128, 4 * KD * 128], BF16, kind="ExternalInput")
    maskb = nc.dram_tensor("maskb", [TC, M], F32, kind="ExternalInput")
    wgenT = nc.dram_tensor("wgenT", [D, 1], BF16, kind="ExternalInput")
    bdec = nc.dram_tensor("bdec", [D, 1], F32, kind="ExternalInput")
    bgen = nc.dram_tensor("bgen", [128, 1], F32, kind="ExternalInput")
    woutT = nc.dram_tensor("woutT", [128, NG, 2 * KD * 128], FP8, kind="ExternalInput")
    da_in = nc.dram_tensor("da_in", [2, 128, KD * RT], FP8, kind="ExternalInput")
    bo = nc.dram_tensor("bo", [128, NVT], F32, kind="ExternalInput")
    ids_f = nc.dram_tensor("ids_f", [B, 4, 128, 1], F32, kind="ExternalInput")
    pidg_f = nc.dram_tensor("pidg_f", [B, 128, 1], F32, kind="ExternalInput")
    pid_loc = nc.dram_tensor("pid_loc", [B, 128, 1], I32, kind="ExternalInput")
    outb = [
        nc.dram_tensor(f"out{b}", [VP, T], BF16, kind="ExternalOutput") for b in range(B)
    ]

    with tile.TileContext(nc) as tc:
        with (
            tc.tile_pool(name="const", bufs=1) as constp,
            tc.tile_pool(name="wa", bufs=4) as wap,            # phase-A weight slabs
            tc.tile_pool(name="wout", bufs=2) as wop,          # W_out fp8 slabs
            tc.tile_pool(name="attn", bufs=1) as ap,
            tc.tile_pool(name="small", bufs=2) as sp,
            tc.tile_pool(name="stash", bufs=64) as stashp,     # z tiles bf16
            tc.tile_pool(name="st8", bufs=3) as stp,           # exp tiles bf16
            tc.tile_pool(name="fin", bufs=4) as finp,          # y out tiles bf16
            tc.tile_pool(name="mrg", bufs=2) as mrgp,
            tc.tile_pool(name="psM", bufs=3, space="PSUM") as psM,   # logits [128,512]
            tc.tile_pool(name="psW", bufs=1, space="PSUM") as psW,   # phase A wide [128,1024]
            tc.tile_pool(name="psA", bufs=2, space="PSUM") as psA,   # misc [128,512]
            tc.tile_pool(name="dram", bufs=1, space="DRAM") as dram,
        ):
            ident = constp.tile([128, 128], F32, tag="ident")
            make_identity(nc, ident[:])
            bos = constp.tile([128, NVT], F32, tag="bos")
            nc.sync.dma_start(out=bos[:], in_=bo[:])
            ones_bf = constp.tile([128, 1], BF16, tag="ones_bf")
            nc.vector.memset(ones_bf[:], 1.0)
            ones128 = constp.tile([128, 128], BF16, tag="ones128")
            nc.vector.memset(ones128[:], 1.0)
            ident_bf = constp.tile([128, 128], BF16, tag="ident_bf")
            nc.vector.tensor_copy(out=ident_bf[:], in_=ident[:])

            # round-0/1 dec activations (fp8) + phase-A dec (bf16)
            da = []
            for r in range(2):
                t_ = constp.tile([128, KD * RT], FP8, tag=f"da{r}", name=f"da{r}")
                nc.sync.dma_start(out=t_[:], in_=da_in[r])
                da.append(t_)
            dmt = constp.tile([128, KD * 128], BF16, tag="dmt")
            nc.sync.dma_start(out=dmt[:], in_=dmt_in[:])

            # fixup constants + small phase-A vectors (load early on sync queue)
            idf = [None] * B
            pidgT = [None] * B
            ploc = [None] * B
            for b in range(B):
                idf[b] = constp.tile([128, 4], F32, tag=f"idf{b}", name=f"idf{b}")
                for mc in range(4):
                    nc.sync.dma_start(out=idf[b][:, mc : mc + 1], in_=ids_f[b, mc])
                pidgT[b] = constp.tile([128, 1], F32, tag=f"pidg{b}", name=f"pidg{b}")
                nc.sync.dma_start(out=pidgT[b][:], in_=pidg_f[b])
                ploc[b] = constp.tile([128, 1], I32, tag=f"ploc{b}", name=f"ploc{b}")
                nc.sync.dma_start(out=ploc[b][:], in_=pid_loc[b])
            mbt = constp.tile([128, M], F32, tag="mbt")
            nc.sync.dma_start(out=mbt[:], in_=maskb[:])
            bd = constp.tile([128, KD], F32, tag="bd")
            nc.sync.dma_start(
                out=bd[:], in_=bdec[:].rearrange("(a p) o -> p (a o)", p=128)
            )
            wg = constp.tile([128, KD], BF16, tag="wg")
            nc.sync.dma_start(
                out=wg[:], in_=wgenT[:].rearrange("(a p) o -> p (a o)", p=128)
            )
            bg = constp.tile([128, 1], F32, tag="bg")
            nc.sync.dma_start(out=bg[:], in_=bgen[:])

            # W_out slab stream helper ------------------------------------
            def load_wslab(s, r):
                t_ = wop.tile([128, NSLAB * 2 * KD * 128], FP8, tag="ws", name=f"ws{r}_{s}")
                nc.sync.dma_start(
                    out=t_[:],
                    in_=woutT[:, s * NSLAB : (s + 1) * NSLAB, :].rearrange(
                        "p g x -> p (g x)"
                    ),
                )
                return t_

            def vt_block(r, vt, wslab, sacc, stash):
                base = (vt % 10) // 2 * 2048 + (vt % 2) * 1024
                ps = psM.tile([128, RT], F32, space="PSUM", tag="psm", name=f"ps{r}_{vt}")
                for kp in range(4):
                    nc.tensor.matmul(
                        out=ps[:],
                        lhsT=wslab[:, base + kp * 256 : base + (kp + 1) * 256].rearrange(
                            "p (two c) -> p two c", two=2
                        ),
                        rhs=da[r][:, kp * 1024 : (kp + 1) * 1024].rearrange(
                            "p (two t) -> p two t", two=2
                        ),
                        start=(kp == 0),
                        stop=(kp == 3),
                        perf_mode=DR,
                    )
                st = stp.tile([128, RT], BF16, tag="st", name=f"st{r}_{vt}")
                nc.scalar.activation(
                    out=st[:], in_=ps[:], func=AF.Exp,
                    bias=bos[:, vt : vt + 1], scale=1.0 / WSCALE,
                )
                z = stashp.tile([128, RT], BF16, tag="z", name=f"z{r}_{vt}")
                nc.vector.tensor_scalar(
                    out=z[:], in0=ps[:], scalar1=1.0 / WSCALE,
                    scalar2=bos[:, vt : vt + 1], op0=ALU.mult, op1=ALU.add,
                )
                nc.vector.tensor_tensor(out=sacc[:], in0=sacc[:], in1=st[:], op=ALU.add)
                stash.append(z)

            # =================================================================
            # Round 0 slab group 0+1 first (keeps PE busy from t~5us), then
            # phase A (its weight DMAs were queued above), then the rest.
            # =================================================================
            sacc = []
            for r in range(2):
                t_ = constp.tile([128, RT], F32, tag=f"sacc{r}", name=f"sacc{r}")
                nc.vector.memset(t_[:], 0.0)
                sacc.append(t_)
            stash0, stash1 = [], []

            ws = load_wslab(0, 0)
            ws_next = load_wslab(1, 0)
            for vt in range(0, 10):
                vt_block(0, vt, ws, sacc[0], stash0)

            # ---------------- Phase A: copy-attention for my 128 tokens -------
            # (weight slab loads go on the sync queue here, after wout slabs 0-1)
            wcs = []
            for h in range(2):
                t_ = wap.tile([128, 4 * KD * 128], BF16, tag="wa", name=f"wc{h}")
                nc.sync.dma_start(out=t_[:], in_=wcs_in[h])
                wcs.append(t_)
            membT = wap.tile([128, KD * 512], BF16, tag="wa", name="membT")
            nc.sync.dma_start(out=membT[:], in_=membT_in[:])
            memb = wap.tile([128, 4 * KD * 128], BF16, tag="wa", name="memb")
            nc.sync.dma_start(out=memb[:], in_=memb_in[:])

            # dprojT[d, t] = sum_e W_copy[e, d] * decT[e, t]
            psDP = psW.tile([128, KD * 128], F32, space="PSUM", tag="psw", name="psDP")
            for ke in range(KD):
                sl = wcs[ke // 4]
                for dc in range(KD):
                    nc.tensor.matmul(
                        out=psDP[:, dc * 128 : (dc + 1) * 128],
                        lhsT=sl[:, (ke % 4) * 1024 + dc * 128 : (ke % 4) * 1024 + (dc + 1) * 128],
                        rhs=dmt[:, ke * 128 : (ke + 1) * 128],
                        start=(ke == 0),
                        stop=(ke == KD - 1),
                    )
            dpT = ap.tile([128, KD * 128], BF16, tag="dpT")
            nc.scalar.copy(dpT[:], psDP[:])

            # scores[t, m] = sum_d dprojT[d, t] * membT[d, m]
            scps = psA.tile([128, M], F32, space="PSUM", tag="psa", name="scps")
            for dc in range(KD):
                nc.tensor.matmul(
                    out=scps[:],
                    lhsT=dpT[:, dc * 128 : (dc + 1) * 128],
                    rhs=membT[:, dc * 512 : (dc + 1) * 512],
                    start=(dc == 0),
                    stop=(dc == KD - 1),
                )
            sc = ap.tile([128, M], F32, tag="sc")
            nc.vector.tensor_tensor(out=sc[:], in0=scps[:], in1=mbt[:], op=ALU.add)
            mx = sp.tile([128, 1], F32, tag="mx")
            nc.vector.reduce_max(out=mx[:], in_=sc[:], axis=mybir.AxisListType.X)
            nmx = sp.tile([128, 1], F32, tag="nmx")
            nc.vector.tensor_scalar_mul(nmx[:], mx[:], -1.0)
            esc = ap.tile([128, M], F32, tag="esc")
            sesum = sp.tile([128, 1], F32, tag="sesum")
            nc.scalar.activation(out=esc[:], in_=sc[:], func=AF.Exp, bias=nmx[:, :1])
            nc.vector.reduce_sum(out=sesum[:], in_=esc[:], axis=mybir.AxisListType.X)
            rinv = sp.tile([128, 1], F32, tag="rinv")
            nc.vector.reciprocal(rinv[:], sesum[:])
            attn = ap.tile([128, M], BF16, tag="attn")
            nc.vector.tensor_scalar_mul(attn[:], esc[:], rinv[:, :1])

            # attnT via PE transpose -> [m-part x4, 128 t] bf16
            aT = ap.tile([128, 4 * 128], BF16, tag="aT")
            for mc in range(4):
                tp = psA.tile([128, 128], BF16, space="PSUM", tag="psa", name=f"tpa{mc}")
                nc.tensor.transpose(
                    out=tp[:], in_=attn[:, mc * 128 : (mc + 1) * 128], identity=ident_bf[:]
                )
                nc.scalar.copy(aT[:, mc * 128 : (mc + 1) * 128], tp[:])

            # attn_outT[d, t] = sum_m memb[m, d] * attnT[m, t]  -> bf16
            psAO = psW.tile([128, KD * 128], F32, space="PSUM", tag="psw", name="psAO")
            for mc in range(4):
                for dc in range(KD):
                    nc.tensor.matmul(
                        out=psAO[:, dc * 128 : (dc + 1) * 128],
                        lhsT=memb[:, mc * 1024 + dc * 128 : mc * 1024 + (dc + 1) * 128],
                        rhs=aT[:, mc * 128 : (mc + 1) * 128],
                        start=(mc == 0),
                        stop=(mc == 3),
                    )
            aoT = ap.tile([128, KD * 128], BF16, tag="aoT")
            nc.scalar.copy(aoT[:], psAO[:])

            # interleave: next W_out slab work to keep PE warm
            ws, ws_next = ws_next, load_wslab(2, 0)
            wds = []
            for h in range(4):
                t_ = wap.tile([128, 4 * KD * 128], BF16, tag="wa", name=f"wd{h}")
                nc.sync.dma_start(out=t_[:], in_=wds_in[h])
                wds.append(t_)
            for vt in range(10, 20):
                vt_block(0, vt, ws, sacc[0], stash0)

            # dwaT[d, t] = tanh(sum_e W_decT[e, d] * dec_catT[e, t] + b_dec[d])
            psTH = psW.tile([128, KD * 128], F32, space="PSUM", tag="psw", name="psTH")
            for ec in range(KE):
                sl = wds[ec // 4]
                rhs = (
                    dmt[:, (ec % KD) * 128 : (ec % KD + 1) * 128]
                    if ec < KD
                    else aoT[:, (ec - KD) * 128 : (ec - KD + 1) * 128]
                )
                for dc in range(KD):
                    nc.tensor.matmul(
                        out=psTH[:, dc * 128 : (dc + 1) * 128],
                        lhsT=sl[:, (ec % 4) * 1024 + dc * 128 : (ec % 4) * 1024 + (dc + 1) * 128],
                        rhs=rhs,
                        start=(ec == 0),
                        stop=(ec == KE - 1),
                    )
            th = ap.tile([128, KD * 128], BF16, tag="th")
            for dc in range(KD):
                nc.scalar.activation(
                    out=th[:, dc * 128 : (dc + 1) * 128],
                    in_=psTH[:, dc * 128 : (dc + 1) * 128],
                    func=AF.Tanh,
                    bias=bd[:, dc : dc + 1],
                )

            # z[t] = sum_d W_gen[d] * dwaT[d, t] ; pg = sigmoid(z + b_gen)
            zps = psA.tile([128, 1], F32, space="PSUM", tag="psa", name="zps")
            for dc in range(KD):
                nc.tensor.matmul(
                    out=zps[:, :1],
                    lhsT=th[:, dc * 128 : (dc + 1) * 128],
                    rhs=wg[:, dc : dc + 1],
                    start=(dc == 0),
                    stop=(dc == KD - 1),
                )
            pg = sp.tile([128, 1], F32, tag="pg")
            nc.scalar.activation(
                out=pg[:], in_=zps[:, :1], func=AF.Sigmoid, bias=bg[:, :1]
            )
            # cp = esc * (rinv * (1 - pg))   (f32), then transpose+cast to bf16
            ompg = sp.tile([128, 1], F32, tag="ompg")
            nc.vector.tensor_scalar(
                out=ompg[:], in0=pg[:], scalar1=-1.0, scalar2=1.0, op0=ALU.mult, op1=ALU.add
            )
            s2 = sp.tile([128, 1], F32, tag="s2")
            nc.vector.tensor_tensor(out=s2[:], in0=rinv[:], in1=ompg[:], op=ALU.mult)
            cp = ap.tile([128, M], F32, tag="cp")
            nc.vector.tensor_scalar_mul(cp[:], esc[:], s2[:, :1])

            # AG contribution [M+1, 128] bf16: rows 0..511 cpT, row 512 pg
            ag_in = dram.tile([M + 1, TC], BF16)
            cpT_bf = ap.tile([128, 128], BF16, tag="cpTbf")
            for mc in range(4):
                tp = psA.tile([128, 128], F32, space="PSUM", tag="psa", name=f"tpc{mc}")
                nc.tensor.transpose(
                    out=tp[:], in_=cp[:, mc * 128 : (mc + 1) * 128], identity=ident[:]
                )
                nc.scalar.copy(cpT_bf[:], tp[:])
                nc.scalar.dma_start(
                    out=ag_in[mc * 128 : (mc + 1) * 128, :], in_=cpT_bf[:]
                )
            pgpad = ap.tile([128, 128], F32, tag="pgpad")
            nc.vector.memset(pgpad[:], 0.0)
            nc.vector.tensor_copy(out=pgpad[:, 0:1], in_=pg[:])
            tp = psA.tile([128, 128], F32, space="PSUM", tag="psa", name="tpg")
            nc.tensor.transpose(out=tp[:], in_=pgpad[:], identity=ident[:])
            pgT_bf = sp.tile([1, 128], BF16, tag="pgT")
            nc.scalar.copy(pgT_bf[:], tp[0:1, :])
            nc.scalar.dma_start(out=ag_in[M : M + 1, :], in_=pgT_bf[:])

            ag_out = dram.tile([NCORES * (M + 1), TC], BF16, addr_space="Shared")
            nc.gpsimd.collective_compute(
                "AllGather",
                ALU.bypass,
                replica_groups=[list(range(NCORES))],
                ins=[ag_in[:].opt()],
                outs=[ag_out[:].opt()],
            )

            # pg row (all tokens) + cpT_all [m-part x4, NT] from AG output
            # (scalar/act DMA queue: these wait on the AllGather)
            pgrow_bf = sp.tile([1, NT], BF16, tag="pgrowbf", bufs=1)
            for c in range(NCORES):
                nc.scalar.dma_start(
                    out=pgrow_bf[:, c * TC : (c + 1) * TC],
                    in_=ag_out[c * (M + 1) + M : c * (M + 1) + M + 1, :],
                )
            lpg = sp.tile([1, NT], F32, tag="lpg", bufs=1)
            nc.scalar.activation(out=lpg[:], in_=pgrow_bf[:], func=AF.Ln)
            cpT = constp.tile([128, 4 * NT], BF16, tag="cpT")
            for c in range(NCORES):
                for mc in range(4):
                    nc.scalar.dma_start(
                        out=cpT[:, mc * NT + c * TC : mc * NT + (c + 1) * TC],
                        in_=ag_out[
                            c * (M + 1) + mc * 128 : c * (M + 1) + (mc + 1) * 128, :
                        ],
                    )

            # ---------------- rest of round 0 ------------------------------
            for s in range(2, NSLAB):
                ws, ws_next = ws_next, (load_wslab(s + 1, 0) if s + 1 < NSLAB else load_wslab(0, 1))
                for vt in range(s * 10, (s + 1) * 10):
                    vt_block(0, vt, ws, sacc[0], stash0)

            def round_sums(r):
                sbf = sp.tile([128, RT], BF16, tag="sbf", name=f"sbf{r}")
                nc.vector.tensor_copy(out=sbf[:], in_=sacc[r][:])
                spp = psA.tile([1, RT], F32, space="PSUM", tag="psa", name=f"spp{r}")
                nc.tensor.matmul(
                    out=spp[:], lhsT=ones_bf[:], rhs=sbf[:], start=True, stop=True
                )
                ar_in = dram.tile([1, RT], F32, tag=f"ar_in{r}", name=f"ar_in{r}")
                ar_out = dram.tile(
                    [1, RT], F32, addr_space="Shared", tag=f"ar_out{r}", name=f"ar_out{r}"
                )
                s_ps = sp.tile([1, RT], F32, tag="s_ps", name=f"s_ps{r}")
                nc.vector.tensor_copy(out=s_ps[:], in_=spp[:])
                nc.sync.dma_start(out=ar_in[:], in_=s_ps[:])
                nc.gpsimd.collective_compute(
                    "AllReduce",
                    ALU.add,
                    replica_groups=[list(range(NCORES))],
                    ins=[ar_in[:].opt()],
                    outs=[ar_out[:].opt()],
                )
                return ar_out

            ar0 = round_sums(0)

            # ---------------- round 1 matmul stream -------------------------
            ws, ws_next = ws_next, load_wslab(1, 1)
            for vt in range(0, 10):
                vt_block(1, vt, ws, sacc[1], stash1)

            # ---------------- round 0 finalize (overlaps round 1 PE) --------
            def finalize(r, ar_out, stash):
                s_glob = sp.tile([1, RT], F32, tag="sglob", name=f"sglob{r}")
                nc.scalar.dma_start(out=s_glob[:], in_=ar_out[:])
                lns = sp.tile([1, RT], F32, tag="lns", name=f"lns{r}")
                nc.scalar.activation(out=lns[:], in_=s_glob[:], func=AF.Ln)
                crow = sp.tile([1, RT], BF16, tag="crow", name=f"crow{r}")
                nc.vector.tensor_tensor(
                    out=crow[:],
                    in0=lpg[:, r * RT : (r + 1) * RT],
                    in1=lns[:],
                    op=ALU.subtract,
                )
                zc = finp.tile([128, RT], BF16, tag="y", name=f"zc{r}")
                nc.vector.memset(zc[:], 0.0)
                nc.vector.tensor_copy(out=zc[0:1, :], in_=crow[:])
                psC = psA.tile([128, RT], F32, space="PSUM", tag="psa", name=f"psC{r}")
                nc.tensor.matmul(
                    out=psC[:], lhsT=ones128[:], rhs=zc[:], start=True, stop=True
                )
                Cbf = sp.tile([128, RT], BF16, tag="Cbf", name=f"Cbf{r}")
                nc.scalar.copy(Cbf[:], psC[:])
                for vt in range(NVT):
                    y = finp.tile([128, RT], BF16, tag="y", name=f"y{r}_{vt}")
                    nc.vector.tensor_tensor(
                        out=y[:], in0=stash[vt][:], in1=Cbf[:], op=ALU.add
                    )
                    for bb in range(2):
                        nc.scalar.dma_start(
                            out=outb[2 * r + bb][vt * 128 : (vt + 1) * 128, :],
                            in_=y[:, bb * T : (bb + 1) * T],
                        )

            finalize(0, ar0, stash0)

            ws, ws_next = ws_next, load_wslab(2, 1)
            for vt in range(10, 20):
                vt_block(1, vt, ws, sacc[1], stash1)

            # ---------------- round 0 scatter fixup -------------------------
            def fixup(r):
                for bb in range(2):
                    b = 2 * r + bb
                    # mg[q, t] = sum_j (ids[j] == pidg[q]) * cpT[j, t]
                    idT = mrgp.tile([128, 128], F32, tag="idT", name=f"idT{b}")
                    tp_ = psA.tile([128, 128], F32, space="PSUM", tag="psa", name=f"tpi{b}")
                    nc.tensor.transpose(
                        out=tp_[:],
                        in_=pidgT[b][:, 0:1].to_broadcast([128, 128]),
                        identity=ident[:],
                    )
                    nc.scalar.copy(idT[:], tp_[:])
                    psmg = psA.tile([128, T], F32, space="PSUM", tag="psa", name=f"psm{b}")
                    for mj in range(4):
                        sel = mrgp.tile([128, 128], BF16, tag="sel", name=f"sel{b}_{mj}")
                        nc.vector.tensor_tensor(
                            out=sel[:],
                            in0=idf[b][:, mj : mj + 1].to_broadcast([128, 128]),
                            in1=idT[:],
                            op=ALU.is_equal,
                        )
                        nc.tensor.matmul(
                            out=psmg[:],
                            lhsT=sel[:],
                            rhs=cpT[:, mj * NT + b * T : mj * NT + (b + 1) * T],
                            start=(mj == 0),
                            stop=(mj == 3),
                        )
                    mg = mrgp.tile([128, T], F32, tag="mg", name=f"mg{b}")
                    nc.scalar.copy(mg[:], psmg[:])

                    g = mrgp.tile([128, T], BF16, tag="g", name=f"g{b}")
                    nc.gpsimd.indirect_dma_start(
                        out=g[:],
                        out_offset=None,
                        in_=outb[b][:],
                        in_offset=bass.IndirectOffsetOnAxis(ap=ploc[b][:, :1], axis=0),
                    )
                    gx = mrgp.tile([128, T], F32, tag="gx", name=f"gx{b}")
                    nc.scalar.activation(out=gx[:], in_=g[:], func=AF.Exp)
                    nc.vector.tensor_tensor(out=gx[:], in0=gx[:], in1=mg[:], op=ALU.add)
                    gz = mrgp.tile([128, T], BF16, tag="gz", name=f"gz{b}")
                    nc.scalar.activation(out=gz[:], in_=gx[:], func=AF.Ln)
                    nc.gpsimd.indirect_dma_start(
                        out=outb[b][:],
                        out_offset=bass.IndirectOffsetOnAxis(ap=ploc[b][:, :1], axis=0),
                        in_=gz[:],
                        in_offset=None,
                    )

            fixup(0)

            for s in range(2, NSLAB):
                ws, ws_next = ws_next, (load_wslab(s + 1, 1) if s + 1 < NSLAB else None)
                for vt in range(s * 10, (s + 1) * 10):
                    vt_block(1, vt, ws, sacc[1], stash1)

            ar1 = round_sums(1)
            finalize(1, ar1, stash1)
            fixup(1)
    nc.finalize()
    return nc


_NC_CACHE = {}


def _get_nc():
    if "nc" not in _NC_CACHE:
        _NC_CACHE["nc"] = build_kernel()
    return _NC_CACHE["nc"]


def kernel(
    decoder_output,
    memory_output,
    memory_sequence_length,
    memory_ids,
    W_copy,
    b_copy,
    W_dec,
    b_dec,
    W_gen,
    b_gen,
    W_out,
    b_out,
):
    decoder_output = np.asarray(decoder_output, dtype=np.float32)
    memory_output = np.asarray(memory_output, dtype=np.float32)
    msl = np.asarray(memory_sequence_length).astype(np.int64)
    ids = np.asarray(memory_ids).astype(np.int64)
    W_copy = np.asarray(W_copy, dtype=np.float32)
    W_dec = np.asarray(W_dec, dtype=np.float32)
    W_gen = np.asarray(W_gen, dtype=np.float32)
    b_dec_a = np.asarray(b_dec, dtype=np.float32)
    b_gen_a = np.asarray(b_gen, dtype=np.float32)
    W_out = np.asarray(W_out, dtype=np.float32)
    b_out_a = np.asarray(b_out, dtype=np.float32)
    # NOTE: b_copy drops out: it shifts scores by a per-token constant, which
    # softmax over the memory axis cancels exactly.

    # ---- shared (core-independent) host prep ----
    dec_flat = decoder_output.reshape(NT, D)  # token g = b*T + t
    # da[r, p, kd*512+t] = dec[r*512+t, kd*128+p]  (fp8)
    da_h = np.ascontiguousarray(
        dec_flat.reshape(2, RT, KD, 128).transpose(0, 3, 2, 1).reshape(2, 128, KD * RT)
    ).astype(F8)
    # wcs[h, p, (ke%4)*1024 + dc*128 + c] = W_copy[(4h+ke%4)*128+p, dc*128+c]
    wcs_h = np.ascontiguousarray(
        W_copy.reshape(2, 4, 128, KD * 128).transpose(0, 2, 1, 3).reshape(2, 128, 4096)
    ).astype(BF)
    # wds[j, p, (ec%4)*1024 + dc*128 + c] = W_dec.T[(4j+ec%4)*128+p, dc*128+c]
    wds_h = np.ascontiguousarray(
        W_dec.T.reshape(4, 4, 128, KD * 128).transpose(0, 2, 1, 3).reshape(4, 128, 4096)
    ).astype(BF)
    wgenT = np.ascontiguousarray(W_gen.reshape(1, D).T.astype(BF))  # [D,1]
    bdec_h = np.ascontiguousarray(b_dec_a.reshape(D, 1))
    bgen_h = np.full((128, 1), float(b_gen_a.ravel()[0]), np.float32)
    ids_f_h = np.ascontiguousarray(ids.reshape(B, 4, 128, 1).astype(np.float32))

    in_maps = []
    for c in range(NCORES):
        b = c // 2
        t0 = (c % 2) * TC
        v0 = c * VS
        v1 = min(v0 + VS, V)
        realw = v1 - v0

        dec_my = decoder_output[b, t0 : t0 + TC]  # [TC, D]
        # dmt[p, ke*128+t] = dec_my[t, ke*128+p]
        dmt_h = np.ascontiguousarray(
            dec_my.reshape(128, KD, 128).transpose(2, 1, 0).reshape(128, KD * 128)
        ).astype(BF)
        memb_b = memory_output[b]  # [M, D]
        membT_h = np.ascontiguousarray(
            memb_b.T.reshape(KD, 128, M).transpose(1, 0, 2).reshape(128, KD * M)
        ).astype(BF)
        memb_h = np.ascontiguousarray(
            memb_b.reshape(4, 128, KD * 128).transpose(1, 0, 2).reshape(128, 4 * KD * 128)
        ).astype(BF)
        L = int(msl[b])
        mrow = np.where(np.arange(M) < L, 0.0, MASK_NEG).astype(np.float32)
        maskb_h = np.ascontiguousarray(np.broadcast_to(mrow, (TC, M)))

        # W_out shard: [NG, 128, 2048] fp8, pre-scaled by WSCALE
        wt = np.zeros((VP, D), dtype=np.float32)
        wt[:realw] = W_out[v0:v1] * WSCALE
        woutT_h = np.ascontiguousarray(
            wt.reshape(NG, 2, 128, KD, 128).transpose(4, 0, 1, 3, 2).reshape(128, NG, 2048)
        ).astype(F8)
        bo_pad = np.full(VP, PAD_BIAS, np.float32)
        bo_pad[:realw] = b_out_a[v0:v1]
        bo_h = np.ascontiguousarray(bo_pad.reshape(NVT, 128).T)  # [128, NVT]

        # packed fixup tables: per batch, unique in-shard valid ids
        pidg_h = np.full((B, 128, 1), -1.0, np.float32)
        ploc_h = np.full((B, 128, 1), SENT, np.int32)
        for bb_ in range(B):
            seen_ = []
            sset = set()
            for m_ in range(M):
                gid = int(ids[bb_, m_])
                if m_ < int(msl[bb_]) and v0 <= gid < v1 and gid not in sset:
                    sset.add(gid)
                    seen_.append(gid)
            assert len(seen_) <= 128, f"in-shard id overflow: {len(seen_)}"
            for q, gid in enumerate(seen_):
                pidg_h[bb_, q, 0] = float(gid)
                ploc_h[bb_, q, 0] = gid - v0

        in_maps.append(
            {
                "dmt_in": dmt_h,
                "wcs_in": wcs_h,
                "wds_in": wds_h,
                "membT_in": membT_h,
                "memb_in": memb_h,
                "maskb": maskb_h,
                "wgenT": wgenT,
                "bdec": bdec_h,
                "bgen": bgen_h,
                "woutT": woutT_h,
                "da_in": da_h,
                "bo": bo_h,
                "ids_f": ids_f_h,
                "pidg_f": pidg_h,
                "pid_loc": ploc_h,
            }
        )

    nc = _get_nc()
    import os

    trace = os.environ.get("KERNEL_TRACE") == "1"
    kw = {}
    if trace:
        kw["trace"] = True
        td = os.environ.get("KERNEL_TRACE_DIR")
        if td:
            os.makedirs(td, exist_ok=True)
            kw["tmpdir"] = td
        tcores = os.environ.get("KERNEL_TRACE_CORES")
        if tcores:
            kw["trace_cores"] = [int(x) for x in tcores.split(",")]
    res = run_bass_kernel_spmd(nc, in_maps, core_ids=list(range(NCORES)), **kw)
    global LAST
    LAST = res

    out_full = np.empty((V, B, T), np.float32)
    for c in range(NCORES):
        v0 = c * VS
        v1 = min(v0 + VS, V)
        realw = v1 - v0
        for b in range(B):
            out_full[v0:v1, b, :] = res.results[c][f"out{b}"][:realw].astype(np.float32)
    return np.ascontiguousarray(out_full.transpose(1, 2, 0))


# revision 20
# speedup vs baseline: 2.4790x; 1.0130x over previous
"""Trainium2 Bass kernel for nn_CopyMechanismMixin (copy-mechanism + vocab projection).

Sharding: vocab-parallel across 8 cores for the dominant [1024tok,1024]x[1024,50257]
projection + softmax + scatter; token-parallel (128 tokens per core) for the copy-
attention prologue. Cross-core: one AllGather (copy probs, bf16) and one AllReduce
(softmax denominators, f32) per 512-token round.

Perf structure:
- fp8(e4m3) DoubleRow matmuls for the vocab projection (weights pre-scaled x32,
  un-scaled in the activation that reads PSUM).
- Large-line DMA layouts: W_out streamed in 5 slabs/round of [128,10240] fp8
  (2KB contiguous per partition line); phase-A weights in [128,4096] bf16 slabs.
- Logits z are stashed in bf16; final log-prob y = z + C (C = ln(p_gen)-ln(S))
  is one vector add per tile; no ln over the bulk output.
- Softmax denominators accumulated on the vector engine (frees PE), one
  partition-sum matmul + AllReduce [1,512] per round.
- Output stored bf16 ([VP, 512] per round), converted to f32 on host.
- Scatter fixup: copy-prob mass merged per unique in-shard vocab id via
  selection matmuls into packed [128, T] tiles; one indirect row gather +
  exp/add/ln + one indirect scatter per batch.
"""

import numpy as np
import ml_dtypes

import concourse.bass as bass
import concourse.bacc as bacc
import concourse.mybir as mybir
import concourse.tile as tile
from concourse.bass_utils import run_bass_kernel_spmd
from concourse.masks import make_identity

F32 = mybir.dt.float32
BF16 = mybir.dt.bfloat16
FP8 = mybir.dt.float8e4
I32 = mybir.dt.int32
BF = ml_dtypes.bfloat16
F8 = ml_dtypes.float8_e4m3
AF = mybir.ActivationFunctionType
ALU = mybir.AluOpType
DR = mybir.MatmulPerfMode.DoubleRow

B, T, M, D, V = 4, 256, 512, 1024, 50257
NCORES = 8
VS = -(-V // NCORES)          # 6283 per-core vocab shard
NVT = -(-VS // 128)           # 50 vocab tiles per core
VP = NVT * 128                # 6400 padded shard width
NG = NVT // 2                 # 25 vt-pair groups in woutT layout
SENT = VP - 1                 # sentinel row (pad region)
PAD_BIAS = -30.0              # b_out value for pad rows -> exp ~ 1e-13
MASK_NEG = -30000.0           # additive score mask
WSCALE = 32.0                 # fp8 weight pre-scale
NT = B * T                    # 1024 total tokens
TC = 128                      # tokens per core (attention phase)
KD = D // 128                 # 8 contraction chunks of 128
KE = 2 * D // 128             # 16
RT = 512                      # tokens per round
NSLAB = 5                     # W_out slabs per round (10 vts each)
LAST = None


def build_kernel():
    nc = bacc.Bacc(
        "TRN2",
        target_bir_lowering=False,
        debug=False,
        enable_asserts=False,
        num_devices=NCORES,
    )
    # ---- I/O ----
    dmt_in = nc.dram_tensor("dmt_in", [128, KD * 128], BF16, kind="ExternalInput")
    wcs_in = nc.dram_tensor("wcs_in", [2, 128, 4 * KD * 128], BF16, kind="ExternalInput")
    wds_in = nc.dram_tensor("wds_in", [4, 128, 4 * KD * 128], BF16, kind="ExternalInput")
    membT_in = nc.dram_tensor("membT_in", [128, KD * 512], BF16, kind="ExternalInput")
    memb_in = nc.dram_tensor("memb_in", [128, 4 * KD * 128], BF16, kind="ExternalInput")
    maskb = nc.dram_tensor("maskb", [TC, M], F32, kind="ExternalInput")
    wgenT = nc.dram_tensor("wgenT", [D, 1], BF16, kind="ExternalInput")
    bdec = nc.dram_tensor("bdec", [D, 1], F32, kind="ExternalInput")
    bgen = nc.dram_tensor("bgen", [128, 1], F32, kind="ExternalInput")
    woutT = nc.dram_tensor("woutT", [128, NG, 2 * KD * 128], FP8, kind="ExternalInput")
    da_in = nc.dram_tensor("da_in", [2, 128, KD * RT], FP8, kind="ExternalInput")
    bo = nc.dram_tensor("bo", [128, NVT], F32, kind="ExternalInput")
    ids_f = nc.dram_tensor("ids_f", [B, 4, 128, 1], F32, kind="ExternalInput")
    pidg_f = nc.dram_tensor("pidg_f", [B, 128, 1], F32, kind="ExternalInput")
    pid_loc = nc.dram_tensor("pid_loc", [B, 128, 1], I32, kind="ExternalInput")
    outb = [
        nc.dram_tensor(f"out{b}", [VP, T], BF16, kind="ExternalOutput") for b in range(B)
    ]
    import os
    dbg = os.environ.get("KERNEL_DEBUG") == "1"
    stash_bufs = 64
    if dbg:
        dbg_pg = nc.dram_tensor("dbg_pg", [1, NT], F32, kind="ExternalOutput")
        dbg_sacc = nc.dram_tensor("dbg_sacc", [2, 128, RT], F32, kind="ExternalOutput")
        dbg_spp = nc.dram_tensor("dbg_spp", [2, 1, RT], F32, kind="ExternalOutput")
        dbg_sg = nc.dram_tensor("dbg_sg", [2, 1, RT], F32, kind="ExternalOutput")
        dbg_esc = nc.dram_tensor("dbg_esc", [128, M], F32, kind="ExternalOutput")
        dbg_cp = nc.dram_tensor("dbg_cp", [128, M], F32, kind="ExternalOutput")
        dbg_aoT = nc.dram_tensor("dbg_aoT", [128, KD * 128], BF16, kind="ExternalOutput")
        dbg_cpT = nc.dram_tensor("dbg_cpT", [128, 4 * NT], BF16, kind="ExternalOutput")
        dbg_dpT = nc.dram_tensor("dbg_dpT", [128, KD * 128], BF16, kind="ExternalOutput")

    with tile.TileContext(nc) as tc:
        with (
            tc.tile_pool(name="const", bufs=1) as constp,
            tc.tile_pool(name="wa", bufs=4) as wap,            # phase-A weight slabs
            tc.tile_pool(name="wout", bufs=2) as wop,          # W_out fp8 slabs
            tc.tile_pool(name="attn", bufs=1) as ap,
            tc.tile_pool(name="small", bufs=2) as sp,
            tc.tile_pool(name="stash", bufs=stash_bufs) as stashp,     # z tiles bf16
            tc.tile_pool(name="st8", bufs=3) as stp,           # exp tiles bf16
            tc.tile_pool(name="fin", bufs=4) as finp,          # y out tiles bf16
            tc.tile_pool(name="mrg", bufs=2) as mrgp,
            tc.tile_pool(name="psM", bufs=3, space="PSUM") as psM,   # logits [128,512]
            tc.tile_pool(name="psW", bufs=1, space="PSUM") as psW,   # phase A wide [128,1024]
            tc.tile_pool(name="psA", bufs=2, space="PSUM") as psA,   # misc [128,512]
            tc.tile_pool(name="dram", bufs=1, space="DRAM") as dram,
        ):
            ident = constp.tile([128, 128], F32, tag="ident")
            make_identity(nc, ident[:])
            bos = constp.tile([128, NVT], F32, tag="bos")
            nc.sync.dma_start(out=bos[:], in_=bo[:])
            ones_bf = constp.tile([128, 1], BF16, tag="ones_bf")
            nc.vector.memset(ones_bf[:], 1.0)
            ones128 = constp.tile([128, 128], BF16, tag="ones128")
            nc.vector.memset(ones128[:], 1.0)
            ident_bf = constp.tile([128, 128], BF16, tag="ident_bf")
            nc.vector.tensor_copy(out=ident_bf[:], in_=ident[:])

            # round-0/1 dec activations (fp8) + phase-A dec (bf16)
            da = []
            for r in range(2):
                t_ = constp.tile([128, KD * RT], FP8, tag=f"da{r}", name=f"da{r}")
                nc.sync.dma_start(out=t_[:], in_=da_in[r])
                da.append(t_)
            dmt = constp.tile([128, KD * 128], BF16, tag="dmt")
            nc.sync.dma_start(out=dmt[:], in_=dmt_in[:])

            # fixup constants + small phase-A vectors (load early on sync queue)
            idf = [None] * B
            pidgT = [None] * B
            ploc = [None] * B
            for b in range(B):
                idf[b] = constp.tile([128, 4], F32, tag=f"idf{b}", name=f"idf{b}")
                for mc in range(4):
                    nc.sync.dma_start(out=idf[b][:, mc : mc + 1], in_=ids_f[b, mc])
                pidgT[b] = constp.tile([128, 1], F32, tag=f"pidg{b}", name=f"pidg{b}")
                nc.sync.dma_start(out=pidgT[b][:], in_=pidg_f[b])
                ploc[b] = constp.tile([128, 1], I32, tag=f"ploc{b}", name=f"ploc{b}")
                nc.sync.dma_start(out=ploc[b][:], in_=pid_loc[b])
            mbt = constp.tile([128, M], F32, tag="mbt")
            nc.sync.dma_start(out=mbt[:], in_=maskb[:])
            bd = constp.tile([128, KD], F32, tag="bd")
            nc.sync.dma_start(
                out=bd[:], in_=bdec[:].rearrange("(a p) o -> p (a o)", p=128)
            )
            wg = constp.tile([128, KD], BF16, tag="wg")
            nc.sync.dma_start(
                out=wg[:], in_=wgenT[:].rearrange("(a p) o -> p (a o)", p=128)
            )
            bg = constp.tile([128, 1], F32, tag="bg")
            nc.sync.dma_start(out=bg[:], in_=bgen[:])

            # W_out slab stream helper ------------------------------------
            def load_wslab(s, r):
                t_ = wop.tile([128, NSLAB * 2 * KD * 128], FP8, tag="ws", name=f"ws{r}_{s}")
                nc.sync.dma_start(
                    out=t_[:],
                    in_=woutT[:, s * NSLAB : (s + 1) * NSLAB, :].rearrange(
                        "p g x -> p (g x)"
                    ),
                )
                return t_

            def vt_block(r, vt, wslab, sacc, stash):
                base = (vt % 10) // 2 * 2048 + (vt % 2) * 1024
                ps = psM.tile([128, RT], F32, space="PSUM", tag="psm", name=f"ps{r}_{vt}")
                for kp in range(4):
                    nc.tensor.matmul(
                        out=ps[:],
                        lhsT=wslab[:, base + kp * 256 : base + (kp + 1) * 256].rearrange(
                            "p (two c) -> p two c", two=2
                        ),
                        rhs=da[r][:, kp * 1024 : (kp + 1) * 1024].rearrange(
                            "p (two t) -> p two t", two=2
                        ),
                        start=(kp == 0),
                        stop=(kp == 3),
                        perf_mode=DR,
                    )
                st = stp.tile([128, RT], BF16, tag="st", name=f"st{r}_{vt}")
                nc.scalar.activation(
                    out=st[:], in_=ps[:], func=AF.Exp,
                    bias=bos[:, vt : vt + 1], scale=1.0 / WSCALE,
                )
                z = stashp.tile([128, RT], BF16, tag="z", name=f"z{r}_{vt}")
                nc.vector.tensor_scalar(
                    out=z[:], in0=ps[:], scalar1=1.0 / WSCALE,
                    scalar2=bos[:, vt : vt + 1], op0=ALU.mult, op1=ALU.add,
                )
                nc.vector.tensor_tensor(out=sacc[:], in0=sacc[:], in1=st[:], op=ALU.add)
                stash.append(z)

            # =================================================================
            # Round 0 slab group 0+1 first (keeps PE busy from t~5us), then
            # phase A (its weight DMAs were queued above), then the rest.
            # =================================================================
            sacc = []
            for r in range(2):
                t_ = constp.tile([128, RT], F32, tag=f"sacc{r}", name=f"sacc{r}")
                nc.vector.memset(t_[:], 0.0)
                sacc.append(t_)
            stash0, stash1 = [], []

            ws = load_wslab(0, 0)
            ws_next = load_wslab(1, 0)
            for vt in range(0, 10):
                vt_block(0, vt, ws, sacc[0], stash0)

            # ---------------- Phase A: copy-attention for my 128 tokens -------
            # (weight slab loads go on the sync queue here, after wout slabs 0-1)
            wcs = []
            for h in range(2):
                t_ = wap.tile([128, 4 * KD * 128], BF16, tag="wa", name=f"wc{h}")
                nc.sync.dma_start(out=t_[:], in_=wcs_in[h])
                wcs.append(t_)
            membT = wap.tile([128, KD * 512], BF16, tag="wa", name="membT")
            nc.sync.dma_start(out=membT[:], in_=membT_in[:])
            memb = wap.tile([128, 4 * KD * 128], BF16, tag="wa", name="memb")
            nc.sync.dma_start(out=memb[:], in_=memb_in[:])

            # dprojT[d, t] = sum_e W_copy[e, d] * decT[e, t]
            psDP = psW.tile([128, KD * 128], F32, space="PSUM", tag="psw", name="psDP")
            for dc in range(KD):
                for ke in range(KD):
                    sl = wcs[ke // 4]
                    nc.tensor.matmul(
                        out=psDP[:, dc * 128 : (dc + 1) * 128],
                        lhsT=sl[:, (ke % 4) * 1024 + dc * 128 : (ke % 4) * 1024 + (dc + 1) * 128],
                        rhs=dmt[:, ke * 128 : (ke + 1) * 128],
                        start=(ke == 0),
                        stop=(ke == KD - 1),
                    )
            dpT = ap.tile([128, KD * 128], BF16, tag="dpT")
            nc.scalar.copy(dpT[:], psDP[:])

            # scores[t, m] = sum_d dprojT[d, t] * membT[d, m]
            scps = psA.tile([128, M], F32, space="PSUM", tag="psa", name="scps")
            for dc in range(KD):
                nc.tensor.matmul(
                    out=scps[:],
                    lhsT=dpT[:, dc * 128 : (dc + 1) * 128],
                    rhs=membT[:, dc * 512 : (dc + 1) * 512],
                    start=(dc == 0),
                    stop=(dc == KD - 1),
                )
            sc = ap.tile([128, M], F32, tag="sc")
            nc.vector.tensor_tensor(out=sc[:], in0=scps[:], in1=mbt[:], op=ALU.add)
            mx = sp.tile([128, 1], F32, tag="mx")
            nc.vector.reduce_max(out=mx[:], in_=sc[:], axis=mybir.AxisListType.X)
            nmx = sp.tile([128, 1], F32, tag="nmx")
            nc.vector.tensor_scalar_mul(nmx[:], mx[:], -1.0)
            esc = ap.tile([128, M], F32, tag="esc")
            sesum = sp.tile([128, 1], F32, tag="sesum")
            nc.scalar.activation(out=esc[:], in_=sc[:], func=AF.Exp, bias=nmx[:, :1])
            nc.vector.reduce_sum(out=sesum[:], in_=esc[:], axis=mybir.AxisListType.X)
            rinv = sp.tile([128, 1], F32, tag="rinv")
            nc.vector.reciprocal(rinv[:], sesum[:])
            attn = ap.tile([128, M], BF16, tag="attn")
            nc.vector.tensor_scalar_mul(attn[:], esc[:], rinv[:, :1])

            # attnT via PE transpose -> [m-part x4, 128 t] bf16
            aT = ap.tile([128, 4 * 128], BF16, tag="aT")
            for mc in range(4):
                tp = psA.tile([128, 128], BF16, space="PSUM", tag="psa", name=f"tpa{mc}")
                nc.tensor.transpose(
                    out=tp[:], in_=attn[:, mc * 128 : (mc + 1) * 128], identity=ident_bf[:]
                )
                nc.scalar.copy(aT[:, mc * 128 : (mc + 1) * 128], tp[:])

            # attn_outT[d, t] = sum_m memb[m, d] * attnT[m, t]  -> bf16
            psAO = psW.tile([128, KD * 128], F32, space="PSUM", tag="psw", name="psAO")
            for dc in range(KD):
                for mc in range(4):
                    nc.tensor.matmul(
                        out=psAO[:, dc * 128 : (dc + 1) * 128],
                        lhsT=memb[:, mc * 1024 + dc * 128 : mc * 1024 + (dc + 1) * 128],
                        rhs=aT[:, mc * 128 : (mc + 1) * 128],
                        start=(mc == 0),
                        stop=(mc == 3),
                    )
            aoT = ap.tile([128, KD * 128], BF16, tag="aoT")
            nc.scalar.copy(aoT[:], psAO[:])

            # interleave: next W_out slab work to keep PE warm
            ws, ws_next = ws_next, load_wslab(2, 0)
            wds = []
            for h in range(4):
                t_ = wap.tile([128, 4 * KD * 128], BF16, tag="wa", name=f"wd{h}")
                nc.sync.dma_start(out=t_[:], in_=wds_in[h])
                wds.append(t_)
            for vt in range(10, 20):
                vt_block(0, vt, ws, sacc[0], stash0)

            # dwaT[d, t] = tanh(sum_e W_decT[e, d] * dec_catT[e, t] + b_dec[d])
            psTH = psW.tile([128, KD * 128], F32, space="PSUM", tag="psw", name="psTH")
            for dc in range(KD):
                for ec in range(KE):
                    sl = wds[ec // 4]
                    rhs = (
                        dmt[:, (ec % KD) * 128 : (ec % KD + 1) * 128]
                        if ec < KD
                        else aoT[:, (ec - KD) * 128 : (ec - KD + 1) * 128]
                    )
                    nc.tensor.matmul(
                        out=psTH[:, dc * 128 : (dc + 1) * 128],
                        lhsT=sl[:, (ec % 4) * 1024 + dc * 128 : (ec % 4) * 1024 + (dc + 1) * 128],
                        rhs=rhs,
                        start=(ec == 0),
                        stop=(ec == KE - 1),
                    )
            th = ap.tile([128, KD * 128], BF16, tag="th")
            for dc in range(KD):
                nc.scalar.activation(
                    out=th[:, dc * 128 : (dc + 1) * 128],
                    in_=psTH[:, dc * 128 : (dc + 1) * 128],
                    func=AF.Tanh,
                    bias=bd[:, dc : dc + 1],
                )

            # z[t] = sum_d W_gen[d] * dwaT[d, t] ; pg = sigmoid(z + b_gen)
            zps = psA.tile([128, 1], F32, space="PSUM", tag="psa", name="zps")
            for dc in range(KD):
                nc.tensor.matmul(
                    out=zps[:, :1],
                    lhsT=th[:, dc * 128 : (dc + 1) * 128],
                    rhs=wg[:, dc : dc + 1],
                    start=(dc == 0),
                    stop=(dc == KD - 1),
                )
            pg = sp.tile([128, 1], F32, tag="pg")
            nc.scalar.activation(
                out=pg[:], in_=zps[:, :1], func=AF.Sigmoid, bias=bg[:, :1]
            )
            # cp = esc * (rinv * (1 - pg))   (f32), then transpose+cast to bf16
            ompg = sp.tile([128, 1], F32, tag="ompg")
            nc.vector.tensor_scalar(
                out=ompg[:], in0=pg[:], scalar1=-1.0, scalar2=1.0, op0=ALU.mult, op1=ALU.add
            )
            s2 = sp.tile([128, 1], F32, tag="s2")
            nc.vector.tensor_tensor(out=s2[:], in0=rinv[:], in1=ompg[:], op=ALU.mult)
            cp = ap.tile([128, M], F32, tag="cp")
            nc.vector.tensor_scalar_mul(cp[:], esc[:], s2[:, :1])
            if dbg:
                nc.sync.dma_start(out=dbg_esc[:], in_=esc[:])
                nc.sync.dma_start(out=dbg_cp[:], in_=cp[:])
                nc.sync.dma_start(out=dbg_aoT[:], in_=aoT[:])
                nc.sync.dma_start(out=dbg_dpT[:], in_=dpT[:])

            # AG contribution [M+1, 128] bf16: rows 0..511 cpT, row 512 pg
            ag_in = dram.tile([M + 1, TC], BF16)
            cpT_bf = ap.tile([128, 128], BF16, tag="cpTbf")
            for mc in range(4):
                tp = psA.tile([128, 128], F32, space="PSUM", tag="psa", name=f"tpc{mc}")
                nc.tensor.transpose(
                    out=tp[:], in_=cp[:, mc * 128 : (mc + 1) * 128], identity=ident[:]
                )
                nc.scalar.copy(cpT_bf[:], tp[:])
                nc.scalar.dma_start(
                    out=ag_in[mc * 128 : (mc + 1) * 128, :], in_=cpT_bf[:]
                )
            pgpad = ap.tile([128, 128], F32, tag="pgpad")
            nc.vector.memset(pgpad[:], 0.0)
            nc.vector.tensor_copy(out=pgpad[:, 0:1], in_=pg[:])
            tp = psA.tile([128, 128], F32, space="PSUM", tag="psa", name="tpg")
            nc.tensor.transpose(out=tp[:], in_=pgpad[:], identity=ident[:])
            pgT_bf = sp.tile([1, 128], BF16, tag="pgT")
            nc.scalar.copy(pgT_bf[:], tp[0:1, :])
            nc.scalar.dma_start(out=ag_in[M : M + 1, :], in_=pgT_bf[:])

            ag_out = dram.tile([NCORES * (M + 1), TC], BF16, addr_space="Shared")
            nc.gpsimd.collective_compute(
                "AllGather",
                ALU.bypass,
                replica_groups=[list(range(NCORES))],
                ins=[ag_in[:].opt()],
                outs=[ag_out[:].opt()],
            )

            # pg row (all tokens) + cpT_all [m-part x4, NT] from AG output
            # (scalar/act DMA queue: these wait on the AllGather)
            pgrow_bf = sp.tile([1, NT], BF16, tag="pgrowbf", bufs=1)
            for c in range(NCORES):
                nc.scalar.dma_start(
                    out=pgrow_bf[:, c * TC : (c + 1) * TC],
                    in_=ag_out[c * (M + 1) + M : c * (M + 1) + M + 1, :],
                )
            lpg = sp.tile([1, NT], F32, tag="lpg", bufs=1)
            nc.scalar.activation(out=lpg[:], in_=pgrow_bf[:], func=AF.Ln)
            if dbg:
                pgf = sp.tile([1, NT], F32, tag="pgf", bufs=1)
                nc.vector.tensor_copy(out=pgf[:], in_=pgrow_bf[:])
                nc.scalar.dma_start(out=dbg_pg[:], in_=pgf[:])
            cpT = constp.tile([128, 4 * NT], BF16, tag="cpT")
            for c in range(NCORES):
                for mc in range(4):
                    nc.scalar.dma_start(
                        out=cpT[:, mc * NT + c * TC : mc * NT + (c + 1) * TC],
                        in_=ag_out[
                            c * (M + 1) + mc * 128 : c * (M + 1) + (mc + 1) * 128, :
                        ],
                    )

            if dbg:
                nc.scalar.dma_start(out=dbg_cpT[:], in_=cpT[:])

            # ---------------- rest of round 0 ------------------------------
            for s in range(2, NSLAB):
                ws, ws_next = ws_next, (load_wslab(s + 1, 0) if s + 1 < NSLAB else load_wslab(0, 1))
                for vt in range(s * 10, (s + 1) * 10):
                    vt_block(0, vt, ws, sacc[0], stash0)

            def round_sums(r):
                sbf = sp.tile([128, RT], BF16, tag="sbf", name=f"sbf{r}")
                nc.vector.tensor_copy(out=sbf[:], in_=sacc[r][:])
                spp = psA.tile([1, RT], F32, space="PSUM", tag="psa", name=f"spp{r}")
                nc.tensor.matmul(
                    out=spp[:], lhsT=ones_bf[:], rhs=sbf[:], start=True, stop=True
                )
                ar_in = dram.tile([1, RT], F32, tag=f"ar_in{r}", name=f"ar_in{r}")
                ar_out = dram.tile(
                    [1, RT], F32, addr_space="Shared", tag=f"ar_out{r}", name=f"ar_out{r}"
                )
                s_ps = sp.tile([1, RT], F32, tag="s_ps", name=f"s_ps{r}")
                nc.vector.tensor_copy(out=s_ps[:], in_=spp[:])
                nc.sync.dma_start(out=ar_in[:], in_=s_ps[:])
                if dbg:
                    nc.sync.dma_start(out=dbg_sacc[r], in_=sacc[r][:])
                    nc.sync.dma_start(out=dbg_spp[r], in_=s_ps[:])
                nc.gpsimd.collective_compute(
                    "AllReduce",
                    ALU.add,
                    replica_groups=[list(range(NCORES))],
                    ins=[ar_in[:].opt()],
                    outs=[ar_out[:].opt()],
                )
                return ar_out

            ar0 = round_sums(0)

            # ---------------- round 1 matmul stream -------------------------
            ws, ws_next = ws_next, load_wslab(1, 1)
            for vt in range(0, 10):
                vt_block(1, vt, ws, sacc[1], stash1)

            # ---------------- round 0 finalize (overlaps round 1 PE) --------
            def finalize(r, ar_out, stash):
                s_glob = sp.tile([1, RT], F32, tag="sglob", name=f"sglob{r}")
                nc.scalar.dma_start(out=s_glob[:], in_=ar_out[:])
                if dbg:
                    nc.scalar.dma_start(out=dbg_sg[r], in_=s_glob[:])
                lns = sp.tile([1, RT], F32, tag="lns", name=f"lns{r}")
                nc.scalar.activation(out=lns[:], in_=s_glob[:], func=AF.Ln)
                crow = sp.tile([1, RT], BF16, tag="crow", name=f"crow{r}")
                nc.vector.tensor_tensor(
                    out=crow[:],
                    in0=lpg[:, r * RT : (r + 1) * RT],
                    in1=lns[:],
                    op=ALU.subtract,
                )
                zc = finp.tile([128, RT], BF16, tag="y", name=f"zc{r}")
                nc.vector.memset(zc[:], 0.0)
                nc.vector.tensor_copy(out=zc[0:1, :], in_=crow[:])
                psC = psA.tile([128, RT], F32, space="PSUM", tag="psa", name=f"psC{r}")
                nc.tensor.matmul(
                    out=psC[:], lhsT=ones128[:], rhs=zc[:], start=True, stop=True
                )
                Cbf = sp.tile([128, RT], BF16, tag="Cbf", name=f"Cbf{r}")
                nc.scalar.copy(Cbf[:], psC[:])
                for vt in range(NVT):
                    y = finp.tile([128, RT], BF16, tag="y", name=f"y{r}_{vt}")
                    nc.vector.tensor_tensor(
                        out=y[:], in0=stash[vt][:], in1=Cbf[:], op=ALU.add
                    )
                    for bb in range(2):
                        nc.scalar.dma_start(
                            out=outb[2 * r + bb][vt * 128 : (vt + 1) * 128, :],
                            in_=y[:, bb * T : (bb + 1) * T],
                        )

            finalize(0, ar0, stash0)

            ws, ws_next = ws_next, load_wslab(2, 1)
            for vt in range(10, 20):
                vt_block(1, vt, ws, sacc[1], stash1)

            # ---------------- round 0 scatter fixup -------------------------
            def fixup(r):
                for bb in range(2):
                    b = 2 * r + bb
                    # mg[q, t] = sum_j (ids[j] == pidg[q]) * cpT[j, t]
                    idT = mrgp.tile([128, 128], F32, tag="idT", name=f"idT{b}")
                    tp_ = psA.tile([128, 128], F32, space="PSUM", tag="psa", name=f"tpi{b}")
                    nc.tensor.transpose(
                        out=tp_[:],
                        in_=pidgT[b][:, 0:1].to_broadcast([128, 128]),
                        identity=ident[:],
                    )
                    nc.scalar.copy(idT[:], tp_[:])
                    psmg = psA.tile([128, T], F32, space="PSUM", tag="psa", name=f"psm{b}")
                    for mj in range(4):
                        sel = mrgp.tile([128, 128], BF16, tag="sel", name=f"sel{b}_{mj}")
                        nc.vector.tensor_tensor(
                            out=sel[:],
                            in0=idf[b][:, mj : mj + 1].to_broadcast([128, 128]),
                            in1=idT[:],
                            op=ALU.is_equal,
                        )
                        nc.tensor.matmul(
                            out=psmg[:],
                            lhsT=sel[:],
                            rhs=cpT[:, mj * NT + b * T : mj * NT + (b + 1) * T],
                            start=(mj == 0),
                            stop=(mj == 3),
                        )
                    mg = mrgp.tile([128, T], F32, tag="mg", name=f"mg{b}")
                    nc.scalar.copy(mg[:], psmg[:])

                    g = mrgp.tile([128, T], BF16, tag="g", name=f"g{b}")
                    nc.gpsimd.indirect_dma_start(
                        out=g[:],
                        out_offset=None,
                        in_=outb[b][:],
                        in_offset=bass.IndirectOffsetOnAxis(ap=ploc[b][:, :1], axis=0),
                    )
                    gx = mrgp.tile([128, T], F32, tag="gx", name=f"gx{b}")
                    nc.scalar.activation(out=gx[:], in_=g[:], func=AF.Exp)
                    nc.vector.tensor_tensor(out=gx[:], in0=gx[:], in1=mg[:], op=ALU.add)
                    gz = mrgp.tile([128, T], BF16, tag="gz", name=f"gz{b}")
                    nc.scalar.activation(out=gz[:], in_=gx[:], func=AF.Ln)
                    nc.gpsimd.indirect_dma_start(
                        out=outb[b][:],
                        out_offset=bass.IndirectOffsetOnAxis(ap=ploc[b][:, :1], axis=0),
                        in_=gz[:],
                        in_offset=None,
                    )

            fixup(0)

            for s in range(2, NSLAB):
                ws, ws_next = ws_next, (load_wslab(s + 1, 1) if s + 1 < NSLAB else None)
                for vt in range(s * 10, (s + 1) * 10):
                    vt_block(1, vt, ws, sacc[1], stash1)

            ar1 = round_sums(1)
            finalize(1, ar1, stash1)
            fixup(1)
    nc.finalize()
    return nc


_NC_CACHE = {}


def _get_nc():
    if "nc" not in _NC_CACHE:
        _NC_CACHE["nc"] = build_kernel()
    return _NC_CACHE["nc"]


def kernel(
    decoder_output,
    memory_output,
    memory_sequence_length,
    memory_ids,
    W_copy,
    b_copy,
    W_dec,
    b_dec,
    W_gen,
    b_gen,
    W_out,
    b_out,
):
    decoder_output = np.asarray(decoder_output, dtype=np.float32)
    memory_output = np.asarray(memory_output, dtype=np.float32)
    msl = np.asarray(memory_sequence_length).astype(np.int64)
    ids = np.asarray(memory_ids).astype(np.int64)
    W_copy = np.asarray(W_copy, dtype=np.float32)
    W_dec = np.asarray(W_dec, dtype=np.float32)
    W_gen = np.asarray(W_gen, dtype=np.float32)
    b_dec_a = np.asarray(b_dec, dtype=np.float32)
    b_gen_a = np.asarray(b_gen, dtype=np.float32)
    W_out = np.asarray(W_out, dtype=np.float32)
    b_out_a = np.asarray(b_out, dtype=np.float32)
    # NOTE: b_copy drops out: it shifts scores by a per-token constant, which
    # softmax over the memory axis cancels exactly.

    # ---- shared (core-independent) host prep ----
    dec_flat = decoder_output.reshape(NT, D)  # token g = b*T + t
    # da[r, p, kd*512+t] = dec[r*512+t, kd*128+p]  (fp8)
    da_h = np.ascontiguousarray(
        dec_flat.reshape(2, RT, KD, 128).transpose(0, 3, 2, 1).reshape(2, 128, KD * RT)
    ).astype(F8)
    # wcs[h, p, (ke%4)*1024 + dc*128 + c] = W_copy[(4h+ke%4)*128+p, dc*128+c]
    wcs_h = np.ascontiguousarray(
        W_copy.reshape(2, 4, 128, KD * 128).transpose(0, 2, 1, 3).reshape(2, 128, 4096)
    ).astype(BF)
    # wds[j, p, (ec%4)*1024 + dc*128 + c] = W_dec.T[(4j+ec%4)*128+p, dc*128+c]
    wds_h = np.ascontiguousarray(
        W_dec.T.reshape(4, 4, 128, KD * 128).transpose(0, 2, 1, 3).reshape(4, 128, 4096)
    ).astype(BF)
    wgenT = np.ascontiguousarray(W_gen.reshape(1, D).T.astype(BF))  # [D,1]
    bdec_h = np.ascontiguousarray(b_dec_a.reshape(D, 1))
    bgen_h = np.full((128, 1), float(b_gen_a.ravel()[0]), np.float32)
    ids_f_h = np.ascontiguousarray(ids.reshape(B, 4, 128, 1).astype(np.float32))

    in_maps = []
    for c in range(NCORES):
        b = c // 2
        t0 = (c % 2) * TC
        v0 = c * VS
        v1 = min(v0 + VS, V)
        realw = v1 - v0

        dec_my = decoder_output[b, t0 : t0 + TC]  # [TC, D]
        # dmt[p, ke*128+t] = dec_my[t, ke*128+p]
        dmt_h = np.ascontiguousarray(
            dec_my.reshape(128, KD, 128).transpose(2, 1, 0).reshape(128, KD * 128)
        ).astype(BF)
        memb_b = memory_output[b]  # [M, D]
        membT_h = np.ascontiguousarray(
            memb_b.T.reshape(KD, 128, M).transpose(1, 0, 2).reshape(128, KD * M)
        ).astype(BF)
        memb_h = np.ascontiguousarray(
            memb_b.reshape(4, 128, KD * 128).transpose(1, 0, 2).reshape(128, 4 * KD * 128)
        ).astype(BF)
        L = int(msl[b])
        mrow = np.where(np.arange(M) < L, 0.0, MASK_NEG).astype(np.float32)
        maskb_h = np.ascontiguousarray(np.broadcast_to(mrow, (TC, M)))

        # W_out shard: [NG, 128, 2048] fp8, pre-scaled by WSCALE
        wt = np.zeros((VP, D), dtype=np.float32)
        wt[:realw] = W_out[v0:v1] * WSCALE
        woutT_h = np.ascontiguousarray(
            wt.reshape(NG, 2, 128, KD, 128).transpose(4, 0, 1, 3, 2).reshape(128, NG, 2048)
        ).astype(F8)
        bo_pad = np.full(VP, PAD_BIAS, np.float32)
        bo_pad[:realw] = b_out_a[v0:v1]
        bo_h = np.ascontiguousarray(bo_pad.reshape(NVT, 128).T)  # [128, NVT]

        # packed fixup tables: per batch, unique in-shard valid ids
        pidg_h = np.full((B, 128, 1), -1.0, np.float32)
        ploc_h = np.full((B, 128, 1), SENT, np.int32)
        for bb_ in range(B):
            seen_ = []
            sset = set()
            for m_ in range(M):
                gid = int(ids[bb_, m_])
                if m_ < int(msl[bb_]) and v0 <= gid < v1 and gid not in sset:
                    sset.add(gid)
                    seen_.append(gid)
            assert len(seen_) <= 128, f"in-shard id overflow: {len(seen_)}"
            for q, gid in enumerate(seen_):
                pidg_h[bb_, q, 0] = float(gid)
                ploc_h[bb_, q, 0] = gid - v0

        in_maps.append(
            {
                "dmt_in": dmt_h,
                "wcs_in": wcs_h,
                "wds_in": wds_h,
                "membT_in": membT_h,
                "memb_in": memb_h,
                "maskb": maskb_h,
                "wgenT": wgenT,
                "bdec": bdec_h,
                "bgen": bgen_h,
                "woutT": woutT_h,
                "da_in": da_h,
                "bo": bo_h,
                "ids_f": ids_f_h,
                "pidg_f": pidg_h,
                "pid_loc": ploc_h,
            }
        )

    nc = _get_nc()
    import os

    trace = os.environ.get("KERNEL_TRACE") == "1"
    kw = {}
    if trace:
        kw["trace"] = True
        td = os.environ.get("KERNEL_TRACE_DIR")
        if td:
            os.makedirs(td, exist_ok=True)
            kw["tmpdir"] = td
        tcores = os.environ.get("KERNEL_TRACE_CORES")
        if tcores:
            kw["trace_cores"] = [int(x) for x in tcores.split(",")]
    res = run_bass_kernel_spmd(nc, in_maps, core_ids=list(range(NCORES)), **kw)
    global LAST
    LAST = res

    out_full = np.empty((V, B, T), np.float32)
    for c in range(NCORES):
        v0 = c * VS
        v1 = min(v0 + VS, V)
        realw = v1 - v0
        for b in range(B):
            out_full[v0:v1, b, :] = res.results[c][f"out{b}"][:realw].astype(np.float32)
    return np.ascontiguousarray(out_full.transpose(1, 2, 0))
